# revision 1
# baseline (speedup 1.0000x reference)
# GCN message-passing kernel for Trainium2 (8 NeuronCores, MPMD).
#
# Math (PyG GCNConv x2 + per-graph MLP readout):
#   A_norm[c,r] = dinv[c] * ew * dinv[r]   (incl. self loops w=1),  dinv = rsqrt(deg)
#   h1 = leaky_relu(A_norm @ x  @ W1 + b1)
#   h2 =            A_norm @ h1 @ W2 + b2
#   z  = reshape(h2, [B, 22*128]);  MLP;  out = tanh(z)*90 + 150
#
# Edges are uniformly random over all nodes (the graphs are NOT closed
# components), so conv2 needs a real device-side gather of h1 rows.
#
# Device plan (3 launches, nodes sharded 22528/core contiguously):
#   L0 (8x same program): deg = windowed-reduce of dest-sorted edge weights
#       (ELL layout), dinv = sqrt(1/deg) on device.
#   host: folds dinv[dst]*ew*dinv[src] into selector strip values; builds
#       per-edge x payloads (conv1 needs only 12B/edge of input data, so it
#       streams sequentially -- no gather).
#   L1 (MPMD x8): conv1: stream slot-x payloads + selector strips; per-chunk
#       matmuls reduce into PSUM per 512-dest group; @W1+b1; PE transpose;
#       leaky -> h1 rows (fp16).
#   L2 (MPMD x8): conv2: dma_gather (int16, 6 source windows of 32768, fp16
#       256B rows, 4 SWDGE queues) -> per-chunk matmuls accumulate into
#       memset PSUM; @W2+b2 -> h2T in SBUF; readout MLP on strided graph
#       slices; tanh*90+150 -> y.

import numpy as np

N = 180224
E = 1441792
HID = 128
NPG = 22
NCORES = 8
NLOC = N // NCORES          # 22528 nodes per core
B = N // NPG                # 8192 graphs
BLOC = B // NCORES          # 1024 graphs per core
GROUP = 512                 # dest columns per PSUM bank group
P = 128
WIN = 32768                 # int16 gather window (rows)
NWIN = (N + WIN - 1) // WIN


# ----------------------------------------------------------------------------
# host-side structure building
# ----------------------------------------------------------------------------

def _sorted_edges(srcs, dsts, ews):
    order = np.argsort(dsts, kind="stable")
    return srcs[order].astype(np.int64), dsts[order].astype(np.int64), \
        ews[order].astype(np.float32)


def _build_conv1(ss, ds, es, c):
    """Dest-major whole-run packing into full 128-slot chunks."""
    d_loc = ds - c * NLOC
    deg = np.bincount(d_loc, minlength=NLOC)
    run_start = np.concatenate([[0], np.cumsum(deg)])[:-1]

    n_groups = (NLOC + GROUP - 1) // GROUP
    chunk_group, chunk_lo, chunk_span = [], [], []
    run_chunk = np.empty(NLOC, np.int64)
    run_slot = np.empty(NLOC, np.int64)
    groups = [[] for _ in range(n_groups)]
    acc, cur, cur_g = 0, -1, -1
    deg_l = deg.tolist()
    for dl in range(NLOC):
        g = dl // GROUP
        L = deg_l[dl]
        if cur < 0 or g != cur_g or acc + L > 128:
            cur = len(chunk_group)
            chunk_group.append(g)
            chunk_lo.append(dl)
            chunk_span.append(0)
            groups[g].append(cur)
            acc, cur_g = 0, g
        run_chunk[dl] = cur
        run_slot[dl] = cur * 128 + acc
        acc += L
        chunk_span[cur] = dl - chunk_lo[cur] + 1

    n_chunks = len(chunk_group)
    chunk_lo = np.asarray(chunk_lo, np.int64)
    chunk_span = np.asarray(chunk_span, np.int64)
    sel_off = np.concatenate([[0], np.cumsum(chunk_span)])
    S = int(sel_off[-1])

    rank = np.arange(len(ss)) - run_start[d_loc]
    slot = run_slot[d_loc] + rank
    slots_src = np.zeros(n_chunks * 128, np.int64)
    slots_src[slot] = ss
    ch_of_e = run_chunk[d_loc]
    sel_row = (slot % 128).astype(np.int64)
    sel_col = sel_off[ch_of_e] + d_loc - chunk_lo[ch_of_e]

    wd = int(deg.max())
    ell = np.zeros((NLOC, wd), np.float32)
    ell[d_loc, rank] = es
    return dict(slots_src=slots_src, sel_row=sel_row, sel_col=sel_col, S=S,
                n_chunks=n_chunks, chunk_lo=chunk_lo, chunk_span=chunk_span,
                sel_off=sel_off, groups=groups,
                ell=ell.reshape(P, NLOC // P, wd), wd=wd,
                d_loc=d_loc, src=ss, ew=es)


def _build_conv2(ss, ds, es, c):
    """(group, window, dest)-sorted slots for windowed int16 gathers.

    Per (g, w) run (padded to x32 slots): one dma_gather batch; chunks are
    32-aligned pieces that never cross a slab column; spans accumulate into
    the group's zeroed PSUM bank (start=False everywhere).
    """
    d_loc = ds - c * NLOC
    g_of = d_loc // GROUP
    w_of = ss // WIN
    order = np.lexsort((d_loc, w_of, g_of))
    s2, d2, e2 = ss[order], d_loc[order], es[order]
    n_groups = (NLOC + GROUP - 1) // GROUP

    key = g_of[order] * NWIN + w_of[order]
    bnd = np.flatnonzero(np.diff(key)) + 1
    starts = np.concatenate([[0], bnd])
    ends = np.concatenate([bnd, [len(key)]])

    slot_src = []
    sel_rows, sel_cols, sel_vals = [], [], []
    d_all, s_all = [], []
    batches = []
    sel_off = 0
    idx_cols = 0
    for st, en in zip(starts, ends):
        g = int(key[st] // NWIN)
        w = int(key[st] % NWIN)
        n_raw = en - st
        n_pad = -n_raw % 32
        n = n_raw + n_pad
        srcs_run = np.concatenate([s2[st:en] - w * WIN,
                                   np.zeros(n_pad, np.int64)])
        dls_run = np.concatenate([d2[st:en], np.full(n_pad, d2[en - 1])])
        vals_run = np.concatenate([e2[st:en], np.zeros(n_pad, np.float32)])
        dglob_run = np.concatenate([d2[st:en] + c * NLOC,
                                    np.full(n_pad, d2[en - 1] + c * NLOC)])
        sglob_run = np.concatenate([s2[st:en], np.full(n_pad, w * WIN)])
        chunks = []
        pos = 0
        while pos < n:
            k = min(128 - (pos % 128), n - pos)
            lo = int(dls_run[pos:pos + k].min())
            hi = int(dls_run[pos:pos + k].max())
            span = hi - lo + 1
            chunks.append(dict(col=pos // 128, base=pos % 128, k=int(k),
                               coff=lo - g * GROUP, span=span, soff=sel_off))
            r = np.arange(pos, pos + k)
            sel_rows.append((r % 128).astype(np.int64))
            sel_cols.append(sel_off + dls_run[pos:pos + k] - lo)
            sel_vals.append(vals_run[pos:pos + k])
            d_all.append(dglob_run[pos:pos + k])
            s_all.append(sglob_run[pos:pos + k])
            sel_off += span
            pos += k
        slot_src.append(srcs_run)
        batches.append(dict(g=g, w=w, icol=idx_cols, n=int(n),
                            cols=(n + 127) // 128, chunks=chunks))
        idx_cols += n // 16
    return dict(batches=batches, n_groups=n_groups,
                slot_src=np.concatenate(slot_src),
                sel_row=np.concatenate(sel_rows),
                sel_col=np.concatenate(sel_cols),
                sel_val=np.concatenate(sel_vals),
                d_glob=np.concatenate(d_all),
                s_glob=np.concatenate(s_all),
                S2=int(sel_off), idx_cols=int(idx_cols))


def _conv2_arrays(st, dinv):
    idx = np.zeros((P, st["idx_cols"]), np.int16)
    pos = 0
    for b in st["batches"]:
        n = b["n"]
        blk = st["slot_src"][pos:pos + n].astype(np.int16).reshape(n // 16, 16).T
        idx[:, b["icol"]:b["icol"] + n // 16] = np.tile(blk, (8, 1))
        pos += n
    sel = np.zeros((P, st["S2"]), np.float16)
    vals = st["sel_val"] * dinv[st["d_glob"]] * dinv[st["s_glob"]]
    sel[st["sel_row"], st["sel_col"]] = vals.astype(np.float16)
    return idx, sel


# ----------------------------------------------------------------------------
# device programs
# ----------------------------------------------------------------------------

def _bass_mods():
    import concourse.bass as bass
    import concourse.bacc as bacc
    import concourse.tile as tile
    from concourse import mybir
    return bass, bacc, tile, mybir


def build_l0(wd, nloc=None):
    nloc = NLOC if nloc is None else nloc
    bass, bacc, tile, mybir = _bass_mods()
    nc = bacc.Bacc("TRN2", target_bir_lowering=False, debug=False, num_devices=1)
    cols = nloc // P
    ell = nc.dram_tensor("ell", [P, cols * wd], mybir.dt.float32,
                         kind="ExternalInput").ap()
    dinv = nc.dram_tensor("dinv", [nloc], mybir.dt.float32,
                          kind="ExternalOutput").ap()
    with tile.TileContext(nc) as tc:
        with tc.tile_pool(name="sb", bufs=1) as sb:
            ell_t = sb.tile([P, cols, wd], mybir.dt.float32)
            nc.sync.dma_start(ell_t[:], ell.rearrange("p (c w) -> p c w", w=wd))
            deg_t = sb.tile([P, cols], mybir.dt.float32)
            nc.vector.tensor_reduce(deg_t[:], ell_t[:],
                                    axis=mybir.AxisListType.X,
                                    op=mybir.AluOpType.add)
            rec_t = sb.tile([P, cols], mybir.dt.float32)
            nc.vector.reciprocal(rec_t[:], deg_t[:])
            dv_t = sb.tile([P, cols], mybir.dt.float32)
            nc.scalar.activation(dv_t[:], rec_t[:],
                                 mybir.ActivationFunctionType.Sqrt)
            nc.sync.dma_start(dinv.rearrange("(p c) -> p c", p=P), dv_t[:])
    nc.compile()
    return nc


def build_l1(core, nloc=None):
    """conv1: stream slot-x fp16 payloads + fp16 selector strips; no gather."""
    nloc = NLOC if nloc is None else nloc
    bass, bacc, tile, mybir = _bass_mods()
    from concourse.masks import make_identity
    from contextlib import ExitStack

    n_chunks = core["n_chunks"]
    S = core["S"]
    groups = core["groups"]
    chunk_lo = core["chunk_lo"]
    chunk_span = core["chunk_span"]
    sel_off = core["sel_off"]
    n_groups = len(groups)

    nc = bacc.Bacc("TRN2", target_bir_lowering=False, debug=False, num_devices=1)
    f32 = mybir.dt.float32
    f16 = mybir.dt.float16
    sx = nc.dram_tensor("sx", [P, n_chunks * 3], f16, kind="ExternalInput").ap()
    sel = nc.dram_tensor("sel", [P, S], f16, kind="ExternalInput").ap()
    W1 = nc.dram_tensor("W1", [3, HID], f32, kind="ExternalInput").ap()
    b1 = nc.dram_tensor("b1", [HID, 1], f32, kind="ExternalInput").ap()
    h1 = nc.dram_tensor("h1", [nloc, HID], f16, kind="ExternalOutput").ap()

    max_cg = max(len(g) for g in groups)
    max_selw = int(max(sel_off[g[-1] + 1] - sel_off[g[0]] for g in groups))

    with tile.TileContext(nc) as tc, ExitStack() as ctx:
        consts = ctx.enter_context(tc.tile_pool(name="consts", bufs=1))
        sb = ctx.enter_context(tc.tile_pool(name="sb", bufs=3))
        ps = ctx.enter_context(tc.tile_pool(name="ps", bufs=2, space="PSUM"))
        pst = ctx.enter_context(tc.tile_pool(name="pst", bufs=2, space="PSUM"))

        W1_t = consts.tile([3, HID], f32)
        nc.sync.dma_start(W1_t[:], W1[:])
        b1_t = consts.tile([HID, 1], f32)
        nc.sync.dma_start(b1_t[:], b1[:])
        ident = consts.tile([P, P], f32)
        make_identity(nc, ident)

        for g in range(n_groups):
            chs = groups[g]
            cg = len(chs)
            j0 = chs[0]
            so0 = int(sel_off[j0])
            selw = int(sel_off[chs[-1] + 1]) - so0
            gwidth = min(GROUP, nloc - g * GROUP)

            sx_t = sb.tile([P, max_cg * 3], f16, tag="sx")
            nc.sync.dma_start(sx_t[:, :cg * 3], sx[:, j0 * 3:(j0 + cg) * 3])
            sel_t = sb.tile([P, max_selw], f16, tag="sel")
            nc.sync.dma_start(sel_t[:, :selw], sel[:, so0:so0 + selw])

            agg_ps = ps.tile([3, GROUP], f32, tag="agg")
            for jj, j in enumerate(chs):
                span = int(chunk_span[j])
                coff = int(chunk_lo[j]) - g * GROUP
                soff = int(sel_off[j]) - so0
                nc.tensor.matmul(agg_ps[:, coff:coff + span],
                                 lhsT=sx_t[:, jj * 3:jj * 3 + 3],
                                 rhs=sel_t[:, soff:soff + span],
                                 start=True, stop=True)
            agg_sb = sb.tile([3, GROUP], f32, tag="aggsb")
            nc.vector.tensor_copy(agg_sb[:, :gwidth], agg_ps[:, :gwidth])

            h1T_ps = pst.tile([HID, GROUP], f32, tag="h1T")
            nc.tensor.matmul(h1T_ps[:, :gwidth], lhsT=W1_t[:],
                             rhs=agg_sb[:, :gwidth], start=True, stop=True)
            h1T_sb = sb.tile([HID, GROUP], f32, tag="h1Tsb")
            nc.scalar.activation(h1T_sb[:, :gwidth], h1T_ps[:, :gwidth],
                                 mybir.ActivationFunctionType.Identity,
                                 bias=b1_t[:, 0:1], scale=1.0)

            nt = (gwidth + P - 1) // P
            rows_t = sb.tile([P, nt, HID], f16, tag="rows")
            for tt in range(nt):
                tr_ps = ps.tile([P, P], f32, tag="tr")
                nc.tensor.transpose(tr_ps[:], h1T_sb[:, tt * P:(tt + 1) * P],
                                    ident[:])
                a_t = sb.tile([P, P], f32, tag="lk_a")
                nc.scalar.activation(a_t[:], tr_ps[:],
                                     mybir.ActivationFunctionType.Identity)
                c_t = sb.tile([P, P], f32, tag="lk_b")
                nc.scalar.activation(c_t[:], tr_ps[:],
                                     mybir.ActivationFunctionType.Identity,
                                     scale=0.01)
                nc.vector.tensor_tensor(rows_t[:, tt, :], a_t[:], c_t[:],
                                        op=mybir.AluOpType.max)
            out_ap = h1[g * GROUP:g * GROUP + gwidth, :]
            out_ap = out_ap.rearrange("(t p) f -> p t f", p=P)
            nc.sync.dma_start(out_ap, rows_t[:, :nt, :])
    nc.compile()
    return nc


def build_l2(st, nloc=None, bloc=None, n_rows=None):
    """conv2 (windowed fp16 dma_gather + accumulate) + readout MLP."""
    nloc = NLOC if nloc is None else nloc
    bloc = BLOC if bloc is None else bloc
    n_rows = N if n_rows is None else n_rows
    bass, bacc, tile, mybir = _bass_mods()
    from contextlib import ExitStack

    batches = st["batches"]
    n_groups = st["n_groups"]
    S2 = st["S2"]
    idx_cols = st["idx_cols"]

    nc = bacc.Bacc("TRN2", target_bir_lowering=False, debug=False,
                   num_devices=1, num_swdge_queues=4)
    f32 = mybir.dt.float32
    f16 = mybir.dt.float16
    h1f = nc.dram_tensor("h1f", [n_rows, HID], f16, kind="ExternalInput").ap()
    idx = nc.dram_tensor("idx", [P, idx_cols], mybir.dt.int16,
                         kind="ExternalInput").ap()
    sel = nc.dram_tensor("sel", [P, S2], f16, kind="ExternalInput").ap()
    W2 = nc.dram_tensor("W2", [HID, HID], f32, kind="ExternalInput").ap()
    b2 = nc.dram_tensor("b2", [HID, 1], f32, kind="ExternalInput").ap()
    Wf0 = nc.dram_tensor("Wf0", [HID, NPG * HID], f32, kind="ExternalInput").ap()
    bf0 = nc.dram_tensor("bf0", [HID, 1], f32, kind="ExternalInput").ap()
    Wf1 = nc.dram_tensor("Wf1", [HID, HID], f32, kind="ExternalInput").ap()
    bf1 = nc.dram_tensor("bf1", [HID, 1], f32, kind="ExternalInput").ap()
    Wout = nc.dram_tensor("Wout", [HID, 1], f32, kind="ExternalInput").ap()
    bo = nc.dram_tensor("bo", [1, 1], f32, kind="ExternalInput").ap()
    y = nc.dram_tensor("y", [bloc], f32, kind="ExternalOutput").ap()

    max_cols = max(b["cols"] for b in batches)
    g_first, g_last = {}, {}
    for b in batches:
        ch0, ch1 = b["chunks"][0], b["chunks"][-1]
        g = b["g"]
        if g not in g_first:
            g_first[g] = ch0["soff"]
        g_last[g] = ch1["soff"] + ch1["span"]
    max_gsel = max(g_last[g] - g_first[g] for g in g_first)

    by_group = [[] for _ in range(n_groups)]
    for b in batches:
        by_group[b["g"]].append(b)

    with tile.TileContext(nc) as tc, ExitStack() as ctx:
        consts = ctx.enter_context(tc.tile_pool(name="consts", bufs=1))
        big = ctx.enter_context(tc.tile_pool(name="big", bufs=1))
        sb = ctx.enter_context(tc.tile_pool(name="sb", bufs=2))
        slabs = ctx.enter_context(tc.tile_pool(name="slabs", bufs=6))
        ps = ctx.enter_context(tc.tile_pool(name="ps", bufs=2, space="PSUM"))
        pst = ctx.enter_context(tc.tile_pool(name="pst", bufs=2, space="PSUM"))

        W2_t = consts.tile([HID, HID], f32)
        nc.sync.dma_start(W2_t[:], W2[:])
        b2_t = consts.tile([HID, 1], f32)
        nc.sync.dma_start(b2_t[:], b2[:])
        Wf0_t = consts.tile([HID, NPG, HID], f32)
        nc.sync.dma_start(Wf0_t[:], Wf0.rearrange("k (j m) -> k j m", j=NPG))
        bf0_t = consts.tile([HID, 1], f32)
        nc.sync.dma_start(bf0_t[:], bf0[:])
        Wf1_t = consts.tile([HID, HID], f32)
        nc.sync.dma_start(Wf1_t[:], Wf1[:])
        bf1_t = consts.tile([HID, 1], f32)
        nc.sync.dma_start(bf1_t[:], bf1[:])
        Wout_t = consts.tile([HID, 1], f32)
        nc.sync.dma_start(Wout_t[:], Wout[:])
        bo_t = consts.tile([1, 1], f32)
        nc.sync.dma_start(bo_t[:], bo[:])
        bf0b_t = consts.tile([HID, 1], f32)
        nc.vector.tensor_scalar_mul(bf0b_t[:], bf0_t[:], 0.01)
        bf1b_t = consts.tile([HID, 1], f32)
        nc.vector.tensor_scalar_mul(bf1b_t[:], bf1_t[:], 0.01)

        h2T = big.tile([HID, nloc], f32)
        qn = 0
        for g in range(n_groups):
            gwidth = min(GROUP, nloc - g * GROUP)
            gs0 = g_first[g]
            gselw = g_last[g] - gs0
            sel_t = sb.tile([P, max_gsel], f16, tag="sel")
            nc.sync.dma_start(sel_t[:, :gselw], sel[:, gs0:gs0 + gselw])

            agg_ps = ps.tile([HID, GROUP], f32, tag="agg")
            nc.vector.memset(agg_ps[:], 0.0)
            for b in by_group[g]:
                n, w, cols = b["n"], b["w"], b["cols"]
                wsz = min(WIN, n_rows - w * WIN)
                idx_t = slabs.tile([P, max_cols * 8], mybir.dt.int16, tag="idx")
                nc.sync.dma_start(idx_t[:, :n // 16],
                                  idx[:, b["icol"]:b["icol"] + n // 16])
                gat_t = slabs.tile([P, max_cols, HID], f16, tag="gat")
                nc.gpsimd.dma_gather(
                    out_ap=gat_t[:, :cols, :],
                    in_ap=h1f[w * WIN:w * WIN + wsz, :],
                    idxs_ap=idx_t[:, :n // 16],
                    num_idxs=n, num_idxs_reg=n, elem_size=HID,
                    single_packet=False, queue_num=qn)
                qn = (qn + 1) % 4
                for ch in b["chunks"]:
                    k, base, col = ch["k"], ch["base"], ch["col"]
                    so = ch["soff"] - gs0
                    nc.tensor.matmul(
                        agg_ps[:, ch["coff"]:ch["coff"] + ch["span"]],
                        lhsT=gat_t[base:base + k, col, :],
                        rhs=sel_t[base:base + k, so:so + ch["span"]],
                        start=False, stop=True, skip_group_check=True)

            agg_sb = sb.tile([HID, GROUP], f32, tag="aggsb")
            nc.vector.tensor_copy(agg_sb[:, :gwidth], agg_ps[:, :gwidth])
            h2T_ps = pst.tile([HID, GROUP], f32, tag="h2T")
            nc.tensor.matmul(h2T_ps[:, :gwidth], lhsT=W2_t[:],
                             rhs=agg_sb[:, :gwidth], start=True, stop=True)
            nc.scalar.activation(h2T[:, g * GROUP:g * GROUP + gwidth],
                                 h2T_ps[:, :gwidth],
                                 mybir.ActivationFunctionType.Identity,
                                 bias=b2_t[:, 0:1], scale=1.0)

        # readout MLP, feature-major
        GT = 512
        n_gt = (bloc + GT - 1) // GT
        y_sb = big.tile([1, bloc], f32)
        for gt in range(n_gt):
            gw = min(GT, bloc - gt * GT)
            f0_ps = ps.tile([HID, GT], f32, tag="agg")
            for j in range(NPG):
                zT = h2T[:, gt * GT * NPG + j:
                         gt * GT * NPG + j + (gw - 1) * NPG + 1:NPG]
                nc.tensor.matmul(f0_ps[:, :gw], lhsT=Wf0_t[:, j, :], rhs=zT,
                                 start=(j == 0), stop=(j == NPG - 1))
            a_t = sb.tile([HID, GT], f32, tag="f0a")
            nc.scalar.activation(a_t[:, :gw], f0_ps[:, :gw],
                                 mybir.ActivationFunctionType.Identity,
                                 bias=bf0_t[:, 0:1])
            c_t = sb.tile([HID, GT], f32, tag="f0b")
            nc.scalar.activation(c_t[:, :gw], f0_ps[:, :gw],
                                 mybir.ActivationFunctionType.Identity,
                                 bias=bf0b_t[:, 0:1], scale=0.01)
            f0_t = sb.tile([HID, GT], f32, tag="f0m")
            nc.vector.tensor_tensor(f0_t[:, :gw], a_t[:, :gw], c_t[:, :gw],
                                    op=mybir.AluOpType.max)

            f1_ps = pst.tile([HID, GT], f32, tag="h2T")
            nc.tensor.matmul(f1_ps[:, :gw], lhsT=Wf1_t[:], rhs=f0_t[:, :gw],
                             start=True, stop=True)
            a2_t = sb.tile([HID, GT], f32, tag="f1a")
            nc.scalar.activation(a2_t[:, :gw], f1_ps[:, :gw],
                                 mybir.ActivationFunctionType.Identity,
                                 bias=bf1_t[:, 0:1])
            c2_t = sb.tile([HID, GT], f32, tag="f1b")
            nc.scalar.activation(c2_t[:, :gw], f1_ps[:, :gw],
                                 mybir.ActivationFunctionType.Identity,
                                 bias=bf1b_t[:, 0:1], scale=0.01)
            f1_t = sb.tile([HID, GT], f32, tag="f1m")
            nc.vector.tensor_tensor(f1_t[:, :gw], a2_t[:, :gw], c2_t[:, :gw],
                                    op=mybir.AluOpType.max)

            o_ps = ps.tile([1, GT], f32, tag="o")
            nc.tensor.matmul(o_ps[:, :gw], lhsT=Wout_t[:], rhs=f1_t[:, :gw],
                             start=True, stop=True)
            t_t = sb.tile([1, GT], f32, tag="tanh")
            nc.scalar.activation(t_t[:, :gw], o_ps[:, :gw],
                                 mybir.ActivationFunctionType.Tanh,
                                 bias=bo_t[:, 0:1], scale=1.0)
            nc.vector.tensor_scalar(y_sb[:, gt * GT:gt * GT + gw], t_t[:, :gw],
                                    scalar1=90.0, scalar2=150.0,
                                    op0=mybir.AluOpType.mult,
                                    op1=mybir.AluOpType.add)
        nc.sync.dma_start(y.rearrange("(a b) -> a b", a=1), y_sb[:])
    nc.compile()
    return nc


# ----------------------------------------------------------------------------
# MPMD runner (one program per device, concurrent dispatch)
# ----------------------------------------------------------------------------

def _make_runner(nc, device):
    import jax
    import concourse.mybir as mybir
    from concourse.bass2jax import (install_neuronx_cc_hook, _bass_exec_p,
                                    partition_id_tensor)
    install_neuronx_cc_hook()
    in_names, out_names, out_avals, zero_shapes = [], [], [], []
    part_name = nc.partition_id_tensor.name if nc.partition_id_tensor else None
    for alloc in nc.m.functions[0].allocations:
        if not isinstance(alloc, mybir.MemoryLocationSet):
            continue
        name = alloc.memorylocations[0].name
        if alloc.kind == "ExternalInput":
            if name != part_name:
                in_names.append(name)
        elif alloc.kind == "ExternalOutput":
            out_names.append(name)
            shape = tuple(alloc.tensor_shape)
            dtype = mybir.dt.np(alloc.dtype)
            out_avals.append(jax.core.ShapedArray(shape, dtype))
            zero_shapes.append((shape, dtype))
    n_params = len(in_names)
    all_in = list(in_names) + list(out_names)
    if part_name is not None:
        all_in = all_in + [part_name]
    donate = tuple(range(n_params, n_params + len(out_names)))

    def _body(*args):
        operands = list(args)
        if part_name is not None:
            operands.append(partition_id_tensor())
        outs = _bass_exec_p.bind(
            *operands,
            out_avals=tuple(out_avals),
            in_names=tuple(all_in),
            out_names=tuple(out_names),
            lowering_input_output_aliases=(),
            sim_require_finite=True,
            sim_require_nnan=True,
            nc=nc,
        )
        return tuple(outs)

    jitted = jax.jit(_body, donate_argnums=donate, keep_unused=True)
    return dict(jit=jitted, in_names=in_names, out_names=out_names,
                zero_shapes=zero_shapes, device=device)


def _run_mpmd(runners, in_maps):
    import jax
    from concurrent.futures import ThreadPoolExecutor
    handle_args = []
    for r, m in zip(runners, in_maps):
        args = [jax.device_put(np.ascontiguousarray(m[n]), r["device"])
                for n in r["in_names"]]
        args += [jax.device_put(np.zeros(s, d), r["device"])
                 for s, d in r["zero_shapes"]]
        handle_args.append((r, args))
    with ThreadPoolExecutor(max_workers=max(1, len(runners))) as ex:
        handles = list(ex.map(lambda ra: ra[0]["jit"](*ra[1]), handle_args))
    jax.block_until_ready(handles)
    return [{n: np.asarray(h[i]) for i, n in enumerate(r["out_names"])}
            for r, h in zip(runners, handles)]


BENCH = False
LAST_TIMINGS = {}


def _bench_launch(name, runners, in_maps, iters=3):
    import time as _time
    import jax
    dev_args = []
    for r, m in zip(runners, in_maps):
        dev_args.append([jax.device_put(np.ascontiguousarray(m[n]), r["device"])
                         for n in r["in_names"]])
    best = None
    for _ in range(iters):
        packs = []
        for r, args in zip(runners, dev_args):
            zeros = [jax.device_put(np.zeros(s, d), r["device"])
                     for s, d in r["zero_shapes"]]
            jax.block_until_ready(zeros)
            packs.append((r, args, zeros))
        t0 = _time.perf_counter()
        outs = [r["jit"](*args, *zeros) for r, args, zeros in packs]
        jax.block_until_ready(outs)
        dt = _time.perf_counter() - t0
        best = dt if best is None else min(best, dt)
    LAST_TIMINGS[name] = best


# ----------------------------------------------------------------------------
# top-level kernel
# ----------------------------------------------------------------------------

def kernel(x, edge_index, edge_weight, W1, b1, W2, b2,
           Wf0, bf0, Wf1, bf1, Wout, bout):
    import jax

    x = np.asarray(x, np.float32)
    src = np.asarray(edge_index[0], np.int64)
    dst = np.asarray(edge_index[1], np.int64)
    ew = np.asarray(edge_weight, np.float32)

    loops = np.arange(N, dtype=np.int64)
    srcs = np.concatenate([src, loops])
    dsts = np.concatenate([dst, loops])
    ews = np.concatenate([ew, np.ones(N, np.float32)])
    ss, ds, es = _sorted_edges(srcs, dsts, ews)
    bounds = np.searchsorted(ds, np.arange(NCORES + 1) * NLOC)

    c1, c2 = [], []
    wd = 0
    for c in range(NCORES):
        e0, e1 = bounds[c], bounds[c + 1]
        c1.append(_build_conv1(ss[e0:e1], ds[e0:e1], es[e0:e1], c))
        c2.append(_build_conv2(ss[e0:e1], ds[e0:e1], es[e0:e1], c))
        wd = max(wd, c1[-1]["wd"])

    devices = jax.devices()[:NCORES]

    # ---- L0: degrees -> dinv (device) ----
    nc0 = build_l0(wd)
    l0_runners = [_make_runner(nc0, devices[c]) for c in range(NCORES)]
    l0_ins = []
    for st in c1:
        ell = st["ell"]
        if ell.shape[2] < wd:
            ell = np.concatenate(
                [ell, np.zeros((P, NLOC // P, wd - ell.shape[2]), np.float32)],
                axis=2)
        l0_ins.append({"ell": np.ascontiguousarray(ell).reshape(P, -1)})
    res0 = _run_mpmd(l0_runners, l0_ins)
    dinv = np.concatenate([res0[c]["dinv"] for c in range(NCORES)])
    if BENCH:
        _bench_launch("L0", l0_runners, l0_ins)

    # ---- host: fold normalization into selectors + conv1 payloads ----
    l1_ins = []
    for c, st in enumerate(c1):
        vals = st["ew"] * dinv[st["d_loc"] + c * NLOC] * dinv[st["src"]]
        sel = np.zeros((P, st["S"]), np.float16)
        sel[st["sel_row"], st["sel_col"]] = vals.astype(np.float16)
        sx = x[st["slots_src"]].astype(np.float16)
        sx = np.ascontiguousarray(
            sx.reshape(st["n_chunks"], 128, 3).transpose(1, 0, 2)
        ).reshape(P, st["n_chunks"] * 3)
        l1_ins.append(dict(sx=sx, sel=sel,
                           W1=np.asarray(W1, np.float32),
                           b1=np.asarray(b1, np.float32).reshape(HID, 1)))

    # ---- L1: conv1 ----
    l1_runners = [_make_runner(build_l1(st), devices[c])
                  for c, st in enumerate(c1)]
    res1 = _run_mpmd(l1_runners, l1_ins)
    h1_full = np.concatenate([r["h1"] for r in res1], axis=0)  # fp16
    if BENCH:
        _bench_launch("L1", l1_runners, l1_ins)

    # ---- L2: conv2 + readout ----
    Wf0_r = np.asarray(Wf0, np.float32).reshape(NPG, HID, HID)
    Wf0_r = np.ascontiguousarray(Wf0_r.transpose(1, 0, 2)).reshape(HID, NPG * HID)
    l2_runners = [_make_runner(build_l2(st), devices[c])
                  for c, st in enumerate(c2)]
    l2_ins = []
    for c, st in enumerate(c2):
        idx_arr, sel2 = _conv2_arrays(st, dinv)
        l2_ins.append(dict(h1f=h1_full, idx=idx_arr, sel=sel2,
                           W2=np.asarray(W2, np.float32),
                           b2=np.asarray(b2, np.float32).reshape(HID, 1),
                           Wf0=Wf0_r,
                           bf0=np.asarray(bf0, np.float32).reshape(HID, 1),
                           Wf1=np.asarray(Wf1, np.float32),
                           bf1=np.asarray(bf1, np.float32).reshape(HID, 1),
                           Wout=np.asarray(Wout, np.float32).reshape(HID, 1),
                           bo=np.asarray(bout, np.float32).reshape(1, 1)))
    res2 = _run_mpmd(l2_runners, l2_ins)
    if BENCH:
        _bench_launch("L2", l2_runners, l2_ins)
    y = np.concatenate([r["y"] for r in res2]).reshape(B, 1)
    return y



# revision 4
# speedup vs baseline: 199.1087x; 199.1087x over previous
# GCN message-passing kernel for Trainium2 (8 NeuronCores, MPMD).
#
# Math (PyG GCNConv x2 + per-graph MLP readout):
#   A_norm[c,r] = dinv[c] * ew * dinv[r]   (incl. self loops w=1),  dinv = rsqrt(deg)
#   h1 = leaky_relu(A_norm @ x  @ W1 + b1)
#   h2 =            A_norm @ h1 @ W2 + b2
#   z  = reshape(h2, [B, 22*128]);  MLP;  out = tanh(z)*90 + 150
#
# Edges are uniformly random over all nodes (the graphs are NOT closed
# components), so conv2 needs a real device-side gather of h1 rows.
#
# Device plan (2 launches, nodes sharded 22528/core contiguously):
#   host: deg/dinv (weighted in-degree) in numpy; folds dinv[dst]*ew*dinv[src]
#       into selector strip values; builds per-edge x payloads (conv1 needs
#       only 12B/edge of input data, so it streams sequentially -- no gather).
#   L1 (MPMD x8): conv1: stream slot-x payloads + selector strips; per-chunk
#       matmuls reduce into PSUM per 512-dest group; @W1+b1; PE transpose;
#       leaky -> h1 rows (fp16).
#   L2 (MPMD x8): conv2: dma_gather (int16, 6 source windows of 32768, fp16
#       256B rows, 4 SWDGE queues) -> per-chunk matmuls accumulate into
#       memset PSUM; @W2+b2 -> h2T in SBUF; readout MLP on strided graph
#       slices; tanh*90+150 -> y.
#
# With PROFILE=True each launch runs under NRT/NTFF profiling and
# LAST_EXEC_NS[name] records neuron-profile exec_time_ns (max over the 8
# concurrently-launched cores).

import numpy as np

N = 180224
E = 1441792
HID = 128
NPG = 22
NCORES = 8
NLOC = N // NCORES          # 22528 nodes per core
B = N // NPG                # 8192 graphs
BLOC = B // NCORES          # 1024 graphs per core
GROUP = 512                 # dest columns per PSUM bank group
P = 128
WIN = 32768                 # int16 gather window (rows)
NWIN = (N + WIN - 1) // WIN


# ----------------------------------------------------------------------------
# host-side structure building
# ----------------------------------------------------------------------------

def _sorted_edges(srcs, dsts, ews):
    order = np.argsort(dsts, kind="stable")
    return srcs[order].astype(np.int64), dsts[order].astype(np.int64), \
        ews[order].astype(np.float32)


def _build_conv1(ss, ds, es, c):
    """Dest-major whole-run packing into full 128-slot chunks."""
    d_loc = ds - c * NLOC
    deg = np.bincount(d_loc, minlength=NLOC)
    run_start = np.concatenate([[0], np.cumsum(deg)])[:-1]

    n_groups = (NLOC + GROUP - 1) // GROUP
    chunk_group, chunk_lo, chunk_span = [], [], []
    run_chunk = np.empty(NLOC, np.int64)
    run_slot = np.empty(NLOC, np.int64)
    groups = [[] for _ in range(n_groups)]
    acc, cur, cur_g = 0, -1, -1
    deg_l = deg.tolist()
    for dl in range(NLOC):
        g = dl // GROUP
        L = deg_l[dl]
        if cur < 0 or g != cur_g or acc + L > 128:
            cur = len(chunk_group)
            chunk_group.append(g)
            chunk_lo.append(dl)
            chunk_span.append(0)
            groups[g].append(cur)
            acc, cur_g = 0, g
        run_chunk[dl] = cur
        run_slot[dl] = cur * 128 + acc
        acc += L
        chunk_span[cur] = dl - chunk_lo[cur] + 1

    n_chunks = len(chunk_group)
    chunk_lo = np.asarray(chunk_lo, np.int64)
    chunk_span = np.asarray(chunk_span, np.int64)
    sel_off = np.concatenate([[0], np.cumsum(chunk_span)])
    S = int(sel_off[-1])

    rank = np.arange(len(ss)) - run_start[d_loc]
    slot = run_slot[d_loc] + rank
    slots_src = np.zeros(n_chunks * 128, np.int64)
    slots_src[slot] = ss
    ch_of_e = run_chunk[d_loc]
    sel_row = (slot % 128).astype(np.int64)
    sel_col = sel_off[ch_of_e] + d_loc - chunk_lo[ch_of_e]

    return dict(slots_src=slots_src, sel_row=sel_row, sel_col=sel_col, S=S,
                n_chunks=n_chunks, chunk_lo=chunk_lo, chunk_span=chunk_span,
                sel_off=sel_off, groups=groups,
                d_loc=d_loc, src=ss, ew=es)


def _build_conv2(ss, ds, es, c):
    """(group, window, dest)-sorted slots for windowed int16 gathers.

    Per (g, w) run (padded to x32 slots): one dma_gather batch; chunks are
    32-aligned pieces that never cross a slab column; spans accumulate into
    the group's zeroed PSUM bank (start=False everywhere).
    """
    d_loc = ds - c * NLOC
    g_of = d_loc // GROUP
    w_of = ss // WIN
    order = np.lexsort((d_loc, w_of, g_of))
    s2, d2, e2 = ss[order], d_loc[order], es[order]
    n_groups = (NLOC + GROUP - 1) // GROUP

    key = g_of[order] * NWIN + w_of[order]
    bnd = np.flatnonzero(np.diff(key)) + 1
    starts = np.concatenate([[0], bnd])
    ends = np.concatenate([bnd, [len(key)]])

    slot_src = []
    sel_rows, sel_cols, sel_vals = [], [], []
    d_all, s_all = [], []
    batches = []
    sel_off = 0
    idx_cols = 0
    for st, en in zip(starts, ends):
        g = int(key[st] // NWIN)
        w = int(key[st] % NWIN)
        n_raw = en - st
        n_pad = -n_raw % 32
        n = n_raw + n_pad
        srcs_run = np.concatenate([s2[st:en] - w * WIN,
                                   np.zeros(n_pad, np.int64)])
        dls_run = np.concatenate([d2[st:en], np.full(n_pad, d2[en - 1])])
        vals_run = np.concatenate([e2[st:en], np.zeros(n_pad, np.float32)])
        dglob_run = np.concatenate([d2[st:en] + c * NLOC,
                                    np.full(n_pad, d2[en - 1] + c * NLOC)])
        sglob_run = np.concatenate([s2[st:en], np.full(n_pad, w * WIN)])
        chunks = []
        pos = 0
        while pos < n:
            k = min(128 - (pos % 128), n - pos)
            lo = int(dls_run[pos:pos + k].min())
            hi = int(dls_run[pos:pos + k].max())
            span = hi - lo + 1
            chunks.append(dict(col=pos // 128, base=pos % 128, k=int(k),
                               coff=lo - g * GROUP, span=span, soff=sel_off))
            r = np.arange(pos, pos + k)
            sel_rows.append((r % 128).astype(np.int64))
            sel_cols.append(sel_off + dls_run[pos:pos + k] - lo)
            sel_vals.append(vals_run[pos:pos + k])
            d_all.append(dglob_run[pos:pos + k])
            s_all.append(sglob_run[pos:pos + k])
            sel_off += span
            pos += k
        slot_src.append(srcs_run)
        batches.append(dict(g=g, w=w, icol=idx_cols, n=int(n),
                            cols=(n + 127) // 128, chunks=chunks))
        idx_cols += n // 16
    return dict(batches=batches, n_groups=n_groups,
                slot_src=np.concatenate(slot_src),
                sel_row=np.concatenate(sel_rows),
                sel_col=np.concatenate(sel_cols),
                sel_val=np.concatenate(sel_vals),
                d_glob=np.concatenate(d_all),
                s_glob=np.concatenate(s_all),
                S2=int(sel_off), idx_cols=int(idx_cols))


def _conv2_arrays(st, dinv):
    idx = np.zeros((P, st["idx_cols"]), np.int16)
    pos = 0
    for b in st["batches"]:
        n = b["n"]
        blk = st["slot_src"][pos:pos + n].astype(np.int16).reshape(n // 16, 16).T
        idx[:, b["icol"]:b["icol"] + n // 16] = np.tile(blk, (8, 1))
        pos += n
    sel = np.zeros((P, st["S2"]), np.float16)
    vals = st["sel_val"] * dinv[st["d_glob"]] * dinv[st["s_glob"]]
    sel[st["sel_row"], st["sel_col"]] = vals.astype(np.float16)
    return idx, sel


# ----------------------------------------------------------------------------
# device programs
# ----------------------------------------------------------------------------

def _bass_mods():
    import concourse.bass as bass
    import concourse.bacc as bacc
    import concourse.tile as tile
    from concourse import mybir
    return bass, bacc, tile, mybir


def build_l1(core, nloc=None):
    """conv1: stream slot-x fp16 payloads + fp16 selector strips; no gather."""
    nloc = NLOC if nloc is None else nloc
    bass, bacc, tile, mybir = _bass_mods()
    from concourse.masks import make_identity
    from contextlib import ExitStack

    n_chunks = core["n_chunks"]
    S = core["S"]
    groups = core["groups"]
    chunk_lo = core["chunk_lo"]
    chunk_span = core["chunk_span"]
    sel_off = core["sel_off"]
    n_groups = len(groups)

    nc = bacc.Bacc("TRN2", target_bir_lowering=False, debug=False, num_devices=1)
    f32 = mybir.dt.float32
    f16 = mybir.dt.float16
    sx = nc.dram_tensor("sx", [P, n_chunks * 3], f16, kind="ExternalInput").ap()
    sel = nc.dram_tensor("sel", [P, S], f16, kind="ExternalInput").ap()
    W1 = nc.dram_tensor("W1", [3, HID], f32, kind="ExternalInput").ap()
    b1 = nc.dram_tensor("b1", [HID, 1], f32, kind="ExternalInput").ap()
    h1 = nc.dram_tensor("h1", [nloc, HID], f16, kind="ExternalOutput").ap()

    max_cg = max(len(g) for g in groups)
    max_selw = int(max(sel_off[g[-1] + 1] - sel_off[g[0]] for g in groups))

    with tile.TileContext(nc) as tc, ExitStack() as ctx:
        consts = ctx.enter_context(tc.tile_pool(name="consts", bufs=1))
        sb = ctx.enter_context(tc.tile_pool(name="sb", bufs=3))
        ps = ctx.enter_context(tc.tile_pool(name="ps", bufs=2, space="PSUM"))
        pst = ctx.enter_context(tc.tile_pool(name="pst", bufs=2, space="PSUM"))

        W1_t = consts.tile([3, HID], f32)
        nc.sync.dma_start(W1_t[:], W1[:])
        b1_t = consts.tile([HID, 1], f32)
        nc.sync.dma_start(b1_t[:], b1[:])
        ident = consts.tile([P, P], f32)
        make_identity(nc, ident)

        for g in range(n_groups):
            chs = groups[g]
            cg = len(chs)
            j0 = chs[0]
            so0 = int(sel_off[j0])
            selw = int(sel_off[chs[-1] + 1]) - so0
            gwidth = min(GROUP, nloc - g * GROUP)

            sx_t = sb.tile([P, max_cg * 3], f16, tag="sx")
            nc.sync.dma_start(sx_t[:, :cg * 3], sx[:, j0 * 3:(j0 + cg) * 3])
            sel_t = sb.tile([P, max_selw], f16, tag="sel")
            nc.sync.dma_start(sel_t[:, :selw], sel[:, so0:so0 + selw])

            agg_ps = ps.tile([3, GROUP], f32, tag="agg")
            for jj, j in enumerate(chs):
                span = int(chunk_span[j])
                coff = int(chunk_lo[j]) - g * GROUP
                soff = int(sel_off[j]) - so0
                nc.tensor.matmul(agg_ps[:, coff:coff + span],
                                 lhsT=sx_t[:, jj * 3:jj * 3 + 3],
                                 rhs=sel_t[:, soff:soff + span],
                                 start=True, stop=True)
            agg_sb = sb.tile([3, GROUP], f32, tag="aggsb")
            nc.vector.tensor_copy(agg_sb[:, :gwidth], agg_ps[:, :gwidth])

            h1T_ps = pst.tile([HID, GROUP], f32, tag="h1T")
            nc.tensor.matmul(h1T_ps[:, :gwidth], lhsT=W1_t[:],
                             rhs=agg_sb[:, :gwidth], start=True, stop=True)
            h1T_sb = sb.tile([HID, GROUP], f32, tag="h1Tsb")
            nc.scalar.activation(h1T_sb[:, :gwidth], h1T_ps[:, :gwidth],
                                 mybir.ActivationFunctionType.Identity,
                                 bias=b1_t[:, 0:1], scale=1.0)

            nt = (gwidth + P - 1) // P
            rows_t = sb.tile([P, nt, HID], f16, tag="rows")
            for tt in range(nt):
                tr_ps = ps.tile([P, P], f32, tag="tr")
                nc.tensor.transpose(tr_ps[:], h1T_sb[:, tt * P:(tt + 1) * P],
                                    ident[:])
                a_t = sb.tile([P, P], f32, tag="lk_a")
                nc.scalar.activation(a_t[:], tr_ps[:],
                                     mybir.ActivationFunctionType.Identity)
                c_t = sb.tile([P, P], f32, tag="lk_b")
                nc.scalar.activation(c_t[:], tr_ps[:],
                                     mybir.ActivationFunctionType.Identity,
                                     scale=0.01)
                nc.vector.tensor_tensor(rows_t[:, tt, :], a_t[:], c_t[:],
                                        op=mybir.AluOpType.max)
            out_ap = h1[g * GROUP:g * GROUP + gwidth, :]
            out_ap = out_ap.rearrange("(t p) f -> p t f", p=P)
            nc.sync.dma_start(out_ap, rows_t[:, :nt, :])
    nc.compile()
    return nc


def build_l2(st, nloc=None, bloc=None, n_rows=None):
    """conv2 (windowed fp16 dma_gather + accumulate) + readout MLP."""
    nloc = NLOC if nloc is None else nloc
    bloc = BLOC if bloc is None else bloc
    n_rows = N if n_rows is None else n_rows
    bass, bacc, tile, mybir = _bass_mods()
    from contextlib import ExitStack

    batches = st["batches"]
    n_groups = st["n_groups"]
    S2 = st["S2"]
    idx_cols = st["idx_cols"]

    nc = bacc.Bacc("TRN2", target_bir_lowering=False, debug=False,
                   num_devices=1, num_swdge_queues=4)
    f32 = mybir.dt.float32
    f16 = mybir.dt.float16
    h1f = nc.dram_tensor("h1f", [n_rows, HID], f16, kind="ExternalInput").ap()
    idx = nc.dram_tensor("idx", [P, idx_cols], mybir.dt.int16,
                         kind="ExternalInput").ap()
    sel = nc.dram_tensor("sel", [P, S2], f16, kind="ExternalInput").ap()
    W2 = nc.dram_tensor("W2", [HID, HID], f32, kind="ExternalInput").ap()
    b2 = nc.dram_tensor("b2", [HID, 1], f32, kind="ExternalInput").ap()
    Wf0 = nc.dram_tensor("Wf0", [HID, NPG * HID], f32, kind="ExternalInput").ap()
    bf0 = nc.dram_tensor("bf0", [HID, 1], f32, kind="ExternalInput").ap()
    Wf1 = nc.dram_tensor("Wf1", [HID, HID], f32, kind="ExternalInput").ap()
    bf1 = nc.dram_tensor("bf1", [HID, 1], f32, kind="ExternalInput").ap()
    Wout = nc.dram_tensor("Wout", [HID, 1], f32, kind="ExternalInput").ap()
    bo = nc.dram_tensor("bo", [1, 1], f32, kind="ExternalInput").ap()
    y = nc.dram_tensor("y", [bloc], f32, kind="ExternalOutput").ap()

    max_cols = max(b["cols"] for b in batches)
    g_first, g_last = {}, {}
    for b in batches:
        ch0, ch1 = b["chunks"][0], b["chunks"][-1]
        g = b["g"]
        if g not in g_first:
            g_first[g] = ch0["soff"]
        g_last[g] = ch1["soff"] + ch1["span"]
    max_gsel = max(g_last[g] - g_first[g] for g in g_first)

    by_group = [[] for _ in range(n_groups)]
    for b in batches:
        by_group[b["g"]].append(b)

    with tile.TileContext(nc) as tc, ExitStack() as ctx:
        consts = ctx.enter_context(tc.tile_pool(name="consts", bufs=1))
        big = ctx.enter_context(tc.tile_pool(name="big", bufs=1))
        sb = ctx.enter_context(tc.tile_pool(name="sb", bufs=2))
        slabs = ctx.enter_context(tc.tile_pool(name="slabs", bufs=6))
        ps = ctx.enter_context(tc.tile_pool(name="ps", bufs=2, space="PSUM"))
        pst = ctx.enter_context(tc.tile_pool(name="pst", bufs=2, space="PSUM"))

        W2_t = consts.tile([HID, HID], f32)
        nc.sync.dma_start(W2_t[:], W2[:])
        b2_t = consts.tile([HID, 1], f32)
        nc.sync.dma_start(b2_t[:], b2[:])
        Wf0_t = consts.tile([HID, NPG, HID], f32)
        nc.sync.dma_start(Wf0_t[:], Wf0.rearrange("k (j m) -> k j m", j=NPG))
        bf0_t = consts.tile([HID, 1], f32)
        nc.sync.dma_start(bf0_t[:], bf0[:])
        Wf1_t = consts.tile([HID, HID], f32)
        nc.sync.dma_start(Wf1_t[:], Wf1[:])
        bf1_t = consts.tile([HID, 1], f32)
        nc.sync.dma_start(bf1_t[:], bf1[:])
        Wout_t = consts.tile([HID, 1], f32)
        nc.sync.dma_start(Wout_t[:], Wout[:])
        bo_t = consts.tile([1, 1], f32)
        nc.sync.dma_start(bo_t[:], bo[:])
        bf0b_t = consts.tile([HID, 1], f32)
        nc.vector.tensor_scalar_mul(bf0b_t[:], bf0_t[:], 0.01)
        bf1b_t = consts.tile([HID, 1], f32)
        nc.vector.tensor_scalar_mul(bf1b_t[:], bf1_t[:], 0.01)

        h2T = big.tile([HID, nloc], f32)
        qn = 0
        for g in range(n_groups):
            gwidth = min(GROUP, nloc - g * GROUP)
            gs0 = g_first[g]
            gselw = g_last[g] - gs0
            sel_t = sb.tile([P, max_gsel], f16, tag="sel")
            nc.sync.dma_start(sel_t[:, :gselw], sel[:, gs0:gs0 + gselw])

            agg_ps = ps.tile([HID, GROUP], f32, tag="agg")
            nc.vector.memset(agg_ps[:], 0.0)
            for b in by_group[g]:
                n, w, cols = b["n"], b["w"], b["cols"]
                wsz = min(WIN, n_rows - w * WIN)
                idx_t = slabs.tile([P, max_cols * 8], mybir.dt.int16, tag="idx")
                nc.sync.dma_start(idx_t[:, :n // 16],
                                  idx[:, b["icol"]:b["icol"] + n // 16])
                gat_t = slabs.tile([P, max_cols, HID], f16, tag="gat")
                nc.gpsimd.dma_gather(
                    out_ap=gat_t[:, :cols, :],
                    in_ap=h1f[w * WIN:w * WIN + wsz, :],
                    idxs_ap=idx_t[:, :n // 16],
                    num_idxs=n, num_idxs_reg=n, elem_size=HID,
                    single_packet=False, queue_num=qn)
                qn = (qn + 1) % 4
                for ch in b["chunks"]:
                    k, base, col = ch["k"], ch["base"], ch["col"]
                    so = ch["soff"] - gs0
                    nc.tensor.matmul(
                        agg_ps[:, ch["coff"]:ch["coff"] + ch["span"]],
                        lhsT=gat_t[base:base + k, col, :],
                        rhs=sel_t[base:base + k, so:so + ch["span"]],
                        start=False, stop=True, skip_group_check=True)

            agg_sb = sb.tile([HID, GROUP], f32, tag="aggsb")
            nc.vector.tensor_copy(agg_sb[:, :gwidth], agg_ps[:, :gwidth])
            h2T_ps = pst.tile([HID, GROUP], f32, tag="h2T")
            nc.tensor.matmul(h2T_ps[:, :gwidth], lhsT=W2_t[:],
                             rhs=agg_sb[:, :gwidth], start=True, stop=True)
            nc.scalar.activation(h2T[:, g * GROUP:g * GROUP + gwidth],
                                 h2T_ps[:, :gwidth],
                                 mybir.ActivationFunctionType.Identity,
                                 bias=b2_t[:, 0:1], scale=1.0)

        # readout MLP, feature-major
        GT = 512
        n_gt = (bloc + GT - 1) // GT
        y_sb = big.tile([1, bloc], f32)
        for gt in range(n_gt):
            gw = min(GT, bloc - gt * GT)
            f0_ps = ps.tile([HID, GT], f32, tag="agg")
            for j in range(NPG):
                zT = h2T[:, gt * GT * NPG + j:
                         gt * GT * NPG + j + (gw - 1) * NPG + 1:NPG]
                nc.tensor.matmul(f0_ps[:, :gw], lhsT=Wf0_t[:, j, :], rhs=zT,
                                 start=(j == 0), stop=(j == NPG - 1))
            a_t = sb.tile([HID, GT], f32, tag="f0a")
            nc.scalar.activation(a_t[:, :gw], f0_ps[:, :gw],
                                 mybir.ActivationFunctionType.Identity,
                                 bias=bf0_t[:, 0:1])
            c_t = sb.tile([HID, GT], f32, tag="f0b")
            nc.scalar.activation(c_t[:, :gw], f0_ps[:, :gw],
                                 mybir.ActivationFunctionType.Identity,
                                 bias=bf0b_t[:, 0:1], scale=0.01)
            f0_t = sb.tile([HID, GT], f32, tag="f0m")
            nc.vector.tensor_tensor(f0_t[:, :gw], a_t[:, :gw], c_t[:, :gw],
                                    op=mybir.AluOpType.max)

            f1_ps = pst.tile([HID, GT], f32, tag="h2T")
            nc.tensor.matmul(f1_ps[:, :gw], lhsT=Wf1_t[:], rhs=f0_t[:, :gw],
                             start=True, stop=True)
            a2_t = sb.tile([HID, GT], f32, tag="f1a")
            nc.scalar.activation(a2_t[:, :gw], f1_ps[:, :gw],
                                 mybir.ActivationFunctionType.Identity,
                                 bias=bf1_t[:, 0:1])
            c2_t = sb.tile([HID, GT], f32, tag="f1b")
            nc.scalar.activation(c2_t[:, :gw], f1_ps[:, :gw],
                                 mybir.ActivationFunctionType.Identity,
                                 bias=bf1b_t[:, 0:1], scale=0.01)
            f1_t = sb.tile([HID, GT], f32, tag="f1m")
            nc.vector.tensor_tensor(f1_t[:, :gw], a2_t[:, :gw], c2_t[:, :gw],
                                    op=mybir.AluOpType.max)

            o_ps = ps.tile([1, GT], f32, tag="o")
            nc.tensor.matmul(o_ps[:, :gw], lhsT=Wout_t[:], rhs=f1_t[:, :gw],
                             start=True, stop=True)
            t_t = sb.tile([1, GT], f32, tag="tanh")
            nc.scalar.activation(t_t[:, :gw], o_ps[:, :gw],
                                 mybir.ActivationFunctionType.Tanh,
                                 bias=bo_t[:, 0:1], scale=1.0)
            nc.vector.tensor_scalar(y_sb[:, gt * GT:gt * GT + gw], t_t[:, :gw],
                                    scalar1=90.0, scalar2=150.0,
                                    op0=mybir.AluOpType.mult,
                                    op1=mybir.AluOpType.add)
        nc.sync.dma_start(y.rearrange("(a b) -> a b", a=1), y_sb[:])
    nc.compile()
    return nc


# ----------------------------------------------------------------------------
# MPMD runner (one program per device, concurrent dispatch)
# ----------------------------------------------------------------------------

def _make_runner(nc, device):
    import jax
    import concourse.mybir as mybir
    from concourse.bass2jax import (install_neuronx_cc_hook, _bass_exec_p,
                                    partition_id_tensor)
    install_neuronx_cc_hook()
    in_names, out_names, out_avals, zero_shapes = [], [], [], []
    part_name = nc.partition_id_tensor.name if nc.partition_id_tensor else None
    for alloc in nc.m.functions[0].allocations:
        if not isinstance(alloc, mybir.MemoryLocationSet):
            continue
        name = alloc.memorylocations[0].name
        if alloc.kind == "ExternalInput":
            if name != part_name:
                in_names.append(name)
        elif alloc.kind == "ExternalOutput":
            out_names.append(name)
            shape = tuple(alloc.tensor_shape)
            dtype = mybir.dt.np(alloc.dtype)
            out_avals.append(jax.core.ShapedArray(shape, dtype))
            zero_shapes.append((shape, dtype))
    n_params = len(in_names)
    all_in = list(in_names) + list(out_names)
    if part_name is not None:
        all_in = all_in + [part_name]
    donate = tuple(range(n_params, n_params + len(out_names)))

    def _body(*args):
        operands = list(args)
        if part_name is not None:
            operands.append(partition_id_tensor())
        outs = _bass_exec_p.bind(
            *operands,
            out_avals=tuple(out_avals),
            in_names=tuple(all_in),
            out_names=tuple(out_names),
            lowering_input_output_aliases=(),
            sim_require_finite=True,
            sim_require_nnan=True,
            nc=nc,
        )
        return tuple(outs)

    jitted = jax.jit(_body, donate_argnums=donate, keep_unused=True)
    return dict(jit=jitted, nc=nc, in_names=in_names, out_names=out_names,
                zero_shapes=zero_shapes, device=device)


# ----------------------------------------------------------------------------
# NTFF profiling (neuron-profile exec_time_ns per launch, PROFILE=True only)
# ----------------------------------------------------------------------------

_AXON_SO = "/opt/axon/libaxon_pjrt.so"


def _profile_hook():
    import ctypes
    lib = ctypes.CDLL(_AXON_SO)
    if not hasattr(lib, "axon_start_nrt_profile"):
        return None
    lib.axon_start_nrt_profile.argtypes = [ctypes.POINTER(ctypes.c_int64),
                                           ctypes.c_size_t]
    lib.axon_start_nrt_profile.restype = ctypes.c_int64
    lib.axon_stop_nrt_profile.argtypes = [ctypes.c_char_p]
    lib.axon_stop_nrt_profile.restype = ctypes.c_int64
    return lib


def _parse_launch_ntffs(tmpdir, runners, compileds, name):
    """NTFF -> neuron-profile JSON -> gauge exec_time_ns, per core.

    The axon profile ships one NTFF + NEFF pair per executable; executables
    are numbered in compile order, which matches runner order.
    """
    import glob as _glob
    import os
    import re
    import subprocess

    regex = re.compile(
        r"^(?P<fname>.*)-process(?P<proc>\d{6})-executable(?P<exec>\d{6})"
        r"-device(?P<device>\d{6})-execution-?(?P<execution>\d+).ntff$")
    by_exe = {}
    for f in _glob.glob(os.path.join(tmpdir, "*.ntff")):
        m = regex.match(os.path.basename(f))
        if m:
            exe = int(m.group("exec"))
            key = (int(m.group("execution")), f)
            if exe not in by_exe or key > by_exe[exe]:
                by_exe[exe] = key
    exes = sorted(by_exe)
    exec_ns, traces = {}, {}
    if len(exes) != len(runners):
        print(f"profile[{name}]: expected {len(runners)} ntffs, "
              f"got {len(exes)} -- skipping parse")
        return exec_ns, traces
    from gauge import trn_perfetto
    procs = []
    for core, (r, exe) in enumerate(zip(runners, exes)):
        ntff = by_exe[exe][1]
        neff_path = ntff.split("-device")[0] + ".neff"
        json_path = os.path.join(tmpdir, f"k{core}.json")
        p = subprocess.Popen(
            ["neuron-profile", "view", "--ignore-nc-buf-usage",
             "-s", ntff, "-n", neff_path, "--output-format=json",
             f"--output-file={json_path}", "--ignore-dma-trace"],
            cwd=tmpdir,
            stdout=subprocess.DEVNULL, stderr=subprocess.DEVNULL)
        procs.append((core, r, json_path, p))
    for core, r, json_path, p in procs:
        rc = p.wait()
        if rc != 0 or not os.path.exists(json_path):
            print(f"profile[{name}]: neuron-profile failed for core {core}")
            continue
        insts, trace_path, ens, scopes = trn_perfetto.main(
            json=json_path, kernel_dev_mode=True, bass_kernel=r["nc"].m,
            out_path=os.path.join(tmpdir, f"trace_{name}_core{core}.pftrace"),
            title=f"{name}-core{core}")
        exec_ns[core] = ens
        traces[core] = json_path
    return exec_ns, traces


def _run_mpmd_profiled(name, runners, in_maps):
    import jax
    import tempfile
    lib = _profile_hook()
    handle_args, compileds = [], []
    for r, m in zip(runners, in_maps):
        args = [jax.device_put(np.ascontiguousarray(m[n]), r["device"])
                for n in r["in_names"]]
        args += [jax.device_put(np.zeros(s, d), r["device"])
                 for s, d in r["zero_shapes"]]
        jax.block_until_ready(args)
        comp = r["jit"].lower(*args).compile()
        compileds.append(comp)
        handle_args.append((comp, args))
    tmpdir = tempfile.mkdtemp(prefix=f"ntff_{name}_")
    dev_ids = [r["device"].id for r in runners]
    import ctypes
    ids = (ctypes.c_int64 * len(dev_ids))(*dev_ids)
    rc = lib.axon_start_nrt_profile(ids, len(dev_ids))
    if rc != 0:
        raise RuntimeError(f"axon_start_nrt_profile rc={rc}")
    try:
        handles = [comp(*args) for comp, args in handle_args]
        jax.block_until_ready(handles)
    finally:
        nfiles = lib.axon_stop_nrt_profile(tmpdir.encode())
        print(f"profile[{name}]: {nfiles} file(s) -> {tmpdir}")
    exec_ns, traces = _parse_launch_ntffs(tmpdir, runners, compileds, name)
    LAST_EXEC_NS[name] = max(exec_ns.values()) if exec_ns else None
    LAST_EXEC_PER_CORE[name] = exec_ns
    LAST_TRACES[name] = traces
    return [{n: np.asarray(h[i]) for i, n in enumerate(r["out_names"])}
            for r, h in zip(runners, handles)]


def _run_mpmd(runners, in_maps, name=None):
    import jax
    from concurrent.futures import ThreadPoolExecutor
    if PROFILE and name is not None:
        return _run_mpmd_profiled(name, runners, in_maps)
    handle_args = []
    for r, m in zip(runners, in_maps):
        args = [jax.device_put(np.ascontiguousarray(m[n]), r["device"])
                for n in r["in_names"]]
        args += [jax.device_put(np.zeros(s, d), r["device"])
                 for s, d in r["zero_shapes"]]
        handle_args.append((r, args))
    with ThreadPoolExecutor(max_workers=max(1, len(runners))) as ex:
        handles = list(ex.map(lambda ra: ra[0]["jit"](*ra[1]), handle_args))
    jax.block_until_ready(handles)
    return [{n: np.asarray(h[i]) for i, n in enumerate(r["out_names"])}
            for r, h in zip(runners, handles)]


BENCH = False
PROFILE = False
LAST_TIMINGS = {}
LAST_EXEC_NS = {}
LAST_EXEC_PER_CORE = {}
LAST_TRACES = {}


def _bench_launch(name, runners, in_maps, iters=3):
    import time as _time
    import jax
    dev_args = []
    for r, m in zip(runners, in_maps):
        dev_args.append([jax.device_put(np.ascontiguousarray(m[n]), r["device"])
                         for n in r["in_names"]])
    best = None
    for _ in range(iters):
        packs = []
        for r, args in zip(runners, dev_args):
            zeros = [jax.device_put(np.zeros(s, d), r["device"])
                     for s, d in r["zero_shapes"]]
            jax.block_until_ready(zeros)
            packs.append((r, args, zeros))
        t0 = _time.perf_counter()
        outs = [r["jit"](*args, *zeros) for r, args, zeros in packs]
        jax.block_until_ready(outs)
        dt = _time.perf_counter() - t0
        best = dt if best is None else min(best, dt)
    LAST_TIMINGS[name] = best


# ----------------------------------------------------------------------------
# top-level kernel
# ----------------------------------------------------------------------------

def kernel(x, edge_index, edge_weight, W1, b1, W2, b2,
           Wf0, bf0, Wf1, bf1, Wout, bout):
    import jax

    x = np.asarray(x, np.float32)
    src = np.asarray(edge_index[0], np.int64)
    dst = np.asarray(edge_index[1], np.int64)
    ew = np.asarray(edge_weight, np.float32)

    loops = np.arange(N, dtype=np.int64)
    srcs = np.concatenate([src, loops])
    dsts = np.concatenate([dst, loops])
    ews = np.concatenate([ew, np.ones(N, np.float32)])
    ss, ds, es = _sorted_edges(srcs, dsts, ews)
    bounds = np.searchsorted(ds, np.arange(NCORES + 1) * NLOC)

    # weighted in-degree -> dinv, on host (pure data prep)
    deg = np.bincount(ds, weights=es.astype(np.float64), minlength=N)
    dinv = (1.0 / np.sqrt(deg)).astype(np.float32)

    c1, c2 = [], []
    for c in range(NCORES):
        e0, e1 = bounds[c], bounds[c + 1]
        c1.append(_build_conv1(ss[e0:e1], ds[e0:e1], es[e0:e1], c))
        c2.append(_build_conv2(ss[e0:e1], ds[e0:e1], es[e0:e1], c))

    devices = jax.devices()[:NCORES]

    # ---- host: fold normalization into selectors + conv1 payloads ----
    l1_ins = []
    for c, st in enumerate(c1):
        vals = st["ew"] * dinv[st["d_loc"] + c * NLOC] * dinv[st["src"]]
        sel = np.zeros((P, st["S"]), np.float16)
        sel[st["sel_row"], st["sel_col"]] = vals.astype(np.float16)
        sx = x[st["slots_src"]].astype(np.float16)
        sx = np.ascontiguousarray(
            sx.reshape(st["n_chunks"], 128, 3).transpose(1, 0, 2)
        ).reshape(P, st["n_chunks"] * 3)
        l1_ins.append(dict(sx=sx, sel=sel,
                           W1=np.asarray(W1, np.float32),
                           b1=np.asarray(b1, np.float32).reshape(HID, 1)))

    # ---- L1: conv1 ----
    l1_runners = [_make_runner(build_l1(st), devices[c])
                  for c, st in enumerate(c1)]
    res1 = _run_mpmd(l1_runners, l1_ins, name="L1")
    h1_full = np.concatenate([r["h1"] for r in res1], axis=0)  # fp16
    if BENCH:
        _bench_launch("L1", l1_runners, l1_ins)

    # ---- L2: conv2 + readout ----
    Wf0_r = np.asarray(Wf0, np.float32).reshape(NPG, HID, HID)
    Wf0_r = np.ascontiguousarray(Wf0_r.transpose(1, 0, 2)).reshape(HID, NPG * HID)
    l2_runners = [_make_runner(build_l2(st), devices[c])
                  for c, st in enumerate(c2)]
    l2_ins = []
    for c, st in enumerate(c2):
        idx_arr, sel2 = _conv2_arrays(st, dinv)
        l2_ins.append(dict(h1f=h1_full, idx=idx_arr, sel=sel2,
                           W2=np.asarray(W2, np.float32),
                           b2=np.asarray(b2, np.float32).reshape(HID, 1),
                           Wf0=Wf0_r,
                           bf0=np.asarray(bf0, np.float32).reshape(HID, 1),
                           Wf1=np.asarray(Wf1, np.float32),
                           bf1=np.asarray(bf1, np.float32).reshape(HID, 1),
                           Wout=np.asarray(Wout, np.float32).reshape(HID, 1),
                           bo=np.asarray(bout, np.float32).reshape(1, 1)))
    res2 = _run_mpmd(l2_runners, l2_ins, name="L2")
    if BENCH:
        _bench_launch("L2", l2_runners, l2_ins)
    y = np.concatenate([r["y"] for r in res2]).reshape(B, 1)
    return y
